# revision 8
# baseline (speedup 1.0000x reference)
"""Trainium2 Bass kernel for shifted-query cross-attention.

Problem: x [B=4, N=2048, D=512], W_qkv [3*H*DH=1536, D]; H=8 heads, DH=64.
  qkv = x @ W.T; q,k,v per head; q_cross[t] = q[t-1] (q_cross[0]=q[0]);
  out = softmax(q_cross*scale @ k.T) @ v, heads re-concatenated.

Sharding: 8 cores = 4 batches x 2 head-groups (4 heads each). Communication
free: each core gets x[b] and the W rows of its 4 heads, produces
out[b][:, g*256:(g+1)*256].

Per-core plan (all matmuls fp32r, fp32 accumulate in PSUM):
  - transpose x -> xT [D, N] and W-shard -> wT [D, 768] via TensorE identity
    transposes (DMA transpose does not support fp32).
  - proj: qT/kT feature-major [256, N] = wT.T @ xT  (heads pair-packed:
    chunk hp holds head 2hp in partitions 0-63, head 2hp+1 in 64-127);
    v token-major [N, 256] = xT.T @ wT_v, stored with a 65th all-ones
    column per head ([v_h | 1]).
  - scores transposed: ST[j, i] = kT.T @ qcT per 128-token j-chunk, two
    heads row-packed on the PE array (K=64 each, array rows 0-63/64-127).
  - exp on ScalarE straight from PSUM with scale folded in (no max
    subtraction: |scores*scale| <= ~2, exp is safe in fp32).
  - outT[65, i] += [v_h | 1].T @ E accumulated over j-chunks in PSUM;
    row 64 = softmax denominators (free via the ones column).
  - PE-transpose outT back to token-major, multiply by reciprocal row
    sums on VectorE, DMA out.
"""

import os
import sys

for _p in ("/opt/trn_rl_repo", "/root/.axon_site/_ro/trn_rl_repo"):
    if os.path.isdir(_p) and _p not in sys.path:
        sys.path.append(_p)

from contextlib import ExitStack

import numpy as np

import concourse.bass as bass
import concourse.tile as tile
from concourse import bacc, mybir
from concourse.masks import make_identity

B, N, D = 4, 2048, 512
H, DH = 8, 64
SCALE = DH**-0.5
NCORES = 8
HG = H // 2  # heads per core = 4
RV = HG * DH  # 256 v rows / output cols per core

F32 = mybir.dt.float32
F32R = mybir.dt.float32r
EXP = mybir.ActivationFunctionType.Exp


def build_kernel(nc: bass.Bass, n_tok: int = N):
    """Emit the per-core program. n_tok parameterized for small sim tests."""
    NI = n_tok // 128  # 128-token chunks
    NI5 = n_tok // 512  # 512-token chunks

    x_d = nc.dram_tensor("x", [n_tok, D], F32, kind="ExternalInput").ap()
    w_d = nc.dram_tensor("w", [3 * RV, D], F32, kind="ExternalInput").ap()
    o_d = nc.dram_tensor("o", [n_tok, RV], F32, kind="ExternalOutput").ap()

    with tile.TileContext(nc) as tc, ExitStack() as ctx:
        cpool = ctx.enter_context(tc.tile_pool(name="const", bufs=1))
        ident = cpool.tile([128, 128], F32)
        make_identity(nc, ident[:])

        sb = ctx.enter_context(tc.tile_pool(name="persist", bufs=1))
        w_sb = sb.tile([128, 6, D], F32)
        wT = sb.tile([128, 4, 3 * RV], F32R)  # wT[p, kc, r] = w[r, kc*128+p]
        x_sb = sb.tile([128, NI, D], F32)
        xT = sb.tile([128, 4, n_tok], F32R)  # xT[p, kc, i] = x[i, kc*128+p]
        # qcT[p, hp, 1+t] = q[t] for head pair hp; col 0 duplicates q[0]
        qcT = sb.tile([128, 2, n_tok + 8], F32R)
        kT = sb.tile([128, 2, n_tok], F32R)
        # v1[p, jc, ha*65 + dd] = v[jc*128+p, ha*64+dd] for dd<64; 1.0 at dd=64
        v1 = sb.tile([128, NI, HG * 65], F32R)
        out_sb = sb.tile([128, NI, RV], F32)

        # ISA memset can't target float32r; stage f32 ones and cast-copy in
        ones_sb = sb.tile([128, NI, HG, 1], F32)
        nc.any.memset(ones_sb[:], 1.0)
        nc.any.tensor_copy(
            v1[:].rearrange("p n (h e) -> p n h e", e=65)[:, :, :, 64:65],
            ones_sb[:],
        )

        # ---- phase 1: load + transpose x and w ----
        with tc.tile_pool(name="ps_tr", bufs=2, space="PSUM") as ps_tr, tc.tile_pool(
            name="ps_proj", bufs=2, space="PSUM"
        ) as ps_proj:
            for rc in range(6):
                nc.sync.dma_start(w_sb[:, rc, :], w_d[rc * 128 : (rc + 1) * 128, :])
            for ic in range(NI):
                nc.sync.dma_start(x_sb[:, ic, :], x_d[ic * 128 : (ic + 1) * 128, :])

            # batch 4 transposes into one PSUM bank + one wide copy so each
            # downstream matmul operand slice has a single producer (walrus
            # caps sync-waits per lowered LDW+MM instruction)
            for kc in range(4):
                for rg in range(2):  # w row-chunk groups: 0-3 and 4-5
                    rcs = range(4) if rg == 0 else range(4, 6)
                    pt = ps_tr.tile([128, 512], F32, tag="pt")
                    for t, rc in enumerate(rcs):
                        nc.tensor.transpose(
                            pt[:, t * 128 : (t + 1) * 128],
                            w_sb[:, rc, kc * 128 : (kc + 1) * 128],
                            ident[:],
                        )
                    nw = len(rcs) * 128
                    nc.any.tensor_copy(
                        wT[:, kc, rg * 512 : rg * 512 + nw], pt[:, :nw]
                    )
            for kc in range(4):
                for i5 in range(NI5):
                    pt = ps_tr.tile([128, 512], F32, tag="pt")
                    for t in range(4):
                        nc.tensor.transpose(
                            pt[:, t * 128 : (t + 1) * 128],
                            x_sb[:, i5 * 4 + t, kc * 128 : (kc + 1) * 128],
                            ident[:],
                        )
                    nc.any.tensor_copy(
                        xT[:, kc, i5 * 512 : (i5 + 1) * 512], pt[:]
                    )

            # ---- phase 2: q/k projection, feature-major ----
            # m = 0,1: q head pairs (0,1), (2,3); m = 2,3: k head pairs
            for m in range(4):
                for i5 in range(NI5):
                    pp = ps_proj.tile([128, 512], F32, tag="pp")
                    for kc in range(4):
                        nc.tensor.matmul(
                            pp[:],
                            wT[:, kc, m * 128 : (m + 1) * 128],
                            xT[:, kc, i5 * 512 : (i5 + 1) * 512],
                            start=(kc == 0),
                            stop=(kc == 3),
                        )
                    if m < 2:
                        nc.any.tensor_copy(
                            qcT[:, m, 1 + i5 * 512 : 1 + (i5 + 1) * 512], pp[:]
                        )
                        if i5 == 0:
                            nc.any.tensor_copy(qcT[:, m, 0:1], pp[:, 0:1])
                    else:
                        nc.any.tensor_copy(
                            kT[:, m - 2, i5 * 512 : (i5 + 1) * 512], pp[:]
                        )

            # ---- phase 3: v projection, token-major ----
            for jc in range(NI):
                pv = ps_proj.tile([128, 512], F32, tag="pp")
                for kc in range(4):
                    nc.tensor.matmul(
                        pv[:, :RV],
                        xT[:, kc, jc * 128 : (jc + 1) * 128],
                        wT[:, kc, 2 * RV : 3 * RV],
                        start=(kc == 0),
                        stop=(kc == 3),
                    )
                nc.any.tensor_copy(
                    v1[:, jc, :].rearrange("p (h e) -> p h e", e=65)[:, :, 0:64],
                    pv[:, :RV].rearrange("p (h e) -> p h e", e=64),
                )

        # ---- phase 4: attention ----
        with tc.tile_pool(name="ps_st", bufs=2, space="PSUM") as ps_st, tc.tile_pool(
            name="ps_ot", bufs=2, space="PSUM"
        ) as ps_ot, tc.tile_pool(
            name="ps_tro", bufs=2, space="PSUM"
        ) as ps_tro, tc.tile_pool(name="e_sb", bufs=3) as e_pool, tc.tile_pool(
            name="ot_sb", bufs=2
        ) as ot_pool, tc.tile_pool(name="rs", bufs=4) as rs_pool:
            for hp in range(2):
                for i5 in range(NI5):
                    ot_ps = [
                        ps_ot.tile([65, 512], F32, tag="ot", name=f"ot{hp}_{i5}_{h}")
                        for h in range(2)
                    ]
                    for jc in range(NI):
                        st = ps_st.tile([128, 1024], F32, tag="st")
                        # two heads row-packed: head 2hp on array rows 0-63,
                        # head 2hp+1 on rows 64-127
                        nc.tensor.matmul(
                            st[:, 0:512],
                            kT[0:64, hp, jc * 128 : (jc + 1) * 128],
                            qcT[0:64, hp, i5 * 512 : (i5 + 1) * 512],
                            start=True,
                            stop=True,
                            tile_position=(0, 0),
                        )
                        nc.tensor.matmul(
                            st[:, 512:1024],
                            kT[64:128, hp, jc * 128 : (jc + 1) * 128],
                            qcT[64:128, hp, i5 * 512 : (i5 + 1) * 512],
                            start=True,
                            stop=True,
                            tile_position=(64, 0),
                        )
                        et = e_pool.tile([128, 1024], F32R, tag="et")
                        nc.scalar.activation(et[:], st[:], EXP, scale=SCALE)
                        for h in range(2):
                            ha = hp * 2 + h
                            nc.tensor.matmul(
                                ot_ps[h][:],
                                v1[:, jc, ha * 65 : (ha + 1) * 65],
                                et[:, h * 512 : (h + 1) * 512],
                                start=(jc == 0),
                                stop=(jc == NI - 1),
                            )
                    for h in range(2):
                        ha = hp * 2 + h
                        ots = ot_pool.tile([65, 512], F32, tag="ots")
                        nc.vector.tensor_copy(ots[:], ot_ps[h][:])
                        for t in range(4):
                            ic = i5 * 4 + t
                            tr = ps_tro.tile([128, 65], F32, tag="tr")
                            nc.tensor.transpose(
                                tr[:], ots[:, t * 128 : (t + 1) * 128], ident[0:65, 0:65]
                            )
                            rs = rs_pool.tile([128, 1], F32, tag="rs")
                            nc.vector.reciprocal(rs[:], tr[:, 64:65])
                            nc.vector.tensor_scalar_mul(
                                out_sb[:, ic, ha * 64 : (ha + 1) * 64],
                                tr[:, 0:64],
                                rs[:],
                            )

            for ic in range(NI):
                nc.sync.dma_start(o_d[ic * 128 : (ic + 1) * 128, :], out_sb[:, ic, :])

    return nc


def make_nc(n_tok: int = N) -> bass.Bass:
    nc = bacc.Bacc("TRN2", target_bir_lowering=False, debug=False)
    build_kernel(nc, n_tok=n_tok)
    nc.compile()
    return nc


def shard_inputs(x: np.ndarray, W_qkv: np.ndarray) -> list[dict]:
    """Core c = (b, g): b = c // 2, g = c % 2 (heads 4g..4g+3)."""
    in_maps = []
    for c in range(NCORES):
        b, g = divmod(c, 2)
        r0 = g * RV
        w_shard = np.concatenate(
            [
                W_qkv[r0 : r0 + RV],
                W_qkv[512 + r0 : 512 + r0 + RV],
                W_qkv[1024 + r0 : 1024 + r0 + RV],
            ],
            axis=0,
        )
        in_maps.append(
            {
                "x": np.ascontiguousarray(x[b], dtype=np.float32),
                "w": np.ascontiguousarray(w_shard, dtype=np.float32),
            }
        )
    return in_maps


def gather_outputs(results: list[dict]) -> np.ndarray:
    out = np.empty((B, N, H * DH), dtype=np.float32)
    for c in range(NCORES):
        b, g = divmod(c, 2)
        out[b, :, g * RV : (g + 1) * RV] = results[c]["o"]
    return out


_CACHED_NC = None


def kernel(x: np.ndarray, W_qkv: np.ndarray) -> np.ndarray:
    global _CACHED_NC
    from concourse.bass_utils import run_bass_kernel_spmd

    if _CACHED_NC is None:
        _CACHED_NC = make_nc()
    in_maps = shard_inputs(np.asarray(x), np.asarray(W_qkv))
    res = run_bass_kernel_spmd(_CACHED_NC, in_maps, core_ids=list(range(NCORES)))
    return gather_outputs(res.results)


if __name__ == "__main__":
    rng = np.random.default_rng(0)
    x = rng.standard_normal((B, N, D), dtype=np.float32)
    w = (rng.standard_normal((3 * H * DH, D), dtype=np.float32) * 0.02).astype(
        np.float32
    )
    out = kernel(x, w)
    print(out.shape, out.dtype)


# revision 10
# speedup vs baseline: 1.0979x; 1.0979x over previous
"""Trainium2 Bass kernel for shifted-query cross-attention.

Problem: x [B=4, N=2048, D=512], W_qkv [3*H*DH=1536, D]; H=8 heads, DH=64.
  qkv = x @ W.T; q,k,v per head; q_cross[t] = q[t-1] (q_cross[0]=q[0]);
  out = softmax(q_cross*scale @ k.T) @ v, heads re-concatenated.

Sharding: 8 cores = 4 batches x 2 head-groups (4 heads each). Communication
free: each core gets x[b] and the W rows of its 4 heads, produces
out[b][:, g*256:(g+1)*256].

Per-core plan (matmul inputs bf16 -- 1 cyc/row on the PE vs 2 for fp32r --
with fp32 accumulation in PSUM throughout):
  - cast x, W to bf16, transpose to xT [D, N] / wT [D, 768] via TensorE
    identity transposes (DMA transpose does not support fp32; PE transpose
    in bf16 is full rate).
  - proj: qT/kT feature-major [256, N] = wT.T @ xT  (heads pair-packed:
    chunk hp holds head 2hp in partitions 0-63, head 2hp+1 in 64-127);
    v token-major [N, 256] = xT.T @ wT_v, stored with a 65th all-ones
    column per head ([v_h | 1]).
  - scores transposed: ST[j, i] = kT.T @ qcT per 128-token j-chunk, two
    heads row-packed on the PE array (K=64 each, array rows 0-63/64-127).
  - exp on ScalarE straight from PSUM with scale folded in (no max
    subtraction: |scores*scale| <= ~2, exp is safe in fp32).
  - outT[65, i] += [v_h | 1].T @ E accumulated over j-chunks in PSUM;
    row 64 = softmax denominators (free via the ones column).
  - PE-transpose outT back to token-major (fp32), multiply by reciprocal
    row sums on VectorE, DMA out.

All data copies are routed explicitly to VectorE -- nc.any lands them on
ScalarE where a [128,512] copy costs ~0.9us vs ~0.5us, and ScalarE is
needed for the exp stream.
"""

import os
import sys

for _p in ("/opt/trn_rl_repo", "/root/.axon_site/_ro/trn_rl_repo"):
    if os.path.isdir(_p) and _p not in sys.path:
        sys.path.append(_p)

from contextlib import ExitStack

import numpy as np

import concourse.bass as bass
import concourse.tile as tile
from concourse import bacc, mybir
from concourse.masks import make_identity

B, N, D = 4, 2048, 512
H, DH = 8, 64
SCALE = DH**-0.5
NCORES = 8
HG = H // 2  # heads per core = 4
RV = HG * DH  # 256 v rows / output cols per core

F32 = mybir.dt.float32
BF16 = mybir.dt.bfloat16
EXP = mybir.ActivationFunctionType.Exp


def build_kernel(nc: bass.Bass, n_tok: int = N):
    """Emit the per-core program. n_tok parameterized for small sim tests."""
    NI = n_tok // 128  # 128-token chunks
    NI5 = n_tok // 512  # 512-token chunks

    x_d = nc.dram_tensor("x", [n_tok, D], F32, kind="ExternalInput").ap()
    w_d = nc.dram_tensor("w", [3 * RV, D], F32, kind="ExternalInput").ap()
    o_d = nc.dram_tensor("o", [n_tok, RV], F32, kind="ExternalOutput").ap()

    with tile.TileContext(nc) as tc, ExitStack() as ctx:
        cpool = ctx.enter_context(tc.tile_pool(name="const", bufs=1))
        ident = cpool.tile([128, 128], BF16)
        make_identity(nc, ident[:])
        identf = cpool.tile([128, 128], F32)
        make_identity(nc, identf[:])

        sb = ctx.enter_context(tc.tile_pool(name="persist", bufs=1))
        w_sb = sb.tile([128, 6, D], F32)
        wb = sb.tile([128, 6, D], BF16)
        wT = sb.tile([128, 4, 3 * RV], BF16)  # wT[p, kc, r] = w[r, kc*128+p]
        x_sb = sb.tile([128, NI, D], F32)
        xb = sb.tile([128, NI, D], BF16)
        xT = sb.tile([128, 4, n_tok], BF16)  # xT[p, kc, i] = x[i, kc*128+p]
        # qcT[p, hp, 1+t] = q[t] for head pair hp; col 0 duplicates q[0]
        qcT = sb.tile([128, 2, n_tok + 8], BF16)
        kT = sb.tile([128, 2, n_tok], BF16)
        # v1[p, jc, ha*65 + dd] = v[jc*128+p, ha*64+dd] for dd<64; 1.0 at dd=64
        v1 = sb.tile([128, NI, HG * 65], BF16)
        out_sb = sb.tile([128, NI, RV], F32)

        nc.vector.memset(
            v1[:].rearrange("p n (h e) -> p n h e", e=65)[:, :, :, 64:65], 1.0
        )

        # ---- phase 1: load, cast to bf16, transpose x and w ----
        with tc.tile_pool(name="ps_tr", bufs=2, space="PSUM") as ps_tr, tc.tile_pool(
            name="ps_proj", bufs=2, space="PSUM"
        ) as ps_proj:
            for rc in range(6):
                nc.sync.dma_start(w_sb[:, rc, :], w_d[rc * 128 : (rc + 1) * 128, :])
                nc.vector.tensor_copy(wb[:, rc, :], w_sb[:, rc, :])
            for ic in range(NI):
                nc.sync.dma_start(x_sb[:, ic, :], x_d[ic * 128 : (ic + 1) * 128, :])
                nc.vector.tensor_copy(xb[:, ic, :], x_sb[:, ic, :])

            # batch 4 transposes into one PSUM bank + one wide copy so each
            # downstream matmul operand slice has a single producer (walrus
            # caps sync-waits per lowered matmul instruction)
            for kc in range(4):
                for rg in range(2):  # w row-chunk groups: 0-3 and 4-5
                    rcs = range(4) if rg == 0 else range(4, 6)
                    pt = ps_tr.tile([128, 512], BF16, tag="pt")
                    for t, rc in enumerate(rcs):
                        nc.tensor.transpose(
                            pt[:, t * 128 : (t + 1) * 128],
                            wb[:, rc, kc * 128 : (kc + 1) * 128],
                            ident[:],
                        )
                    nw = len(rcs) * 128
                    nc.vector.tensor_copy(
                        wT[:, kc, rg * 512 : rg * 512 + nw], pt[:, :nw]
                    )
            for kc in range(4):
                for i5 in range(NI5):
                    pt = ps_tr.tile([128, 512], BF16, tag="pt")
                    for t in range(4):
                        nc.tensor.transpose(
                            pt[:, t * 128 : (t + 1) * 128],
                            xb[:, i5 * 4 + t, kc * 128 : (kc + 1) * 128],
                            ident[:],
                        )
                    nc.vector.tensor_copy(
                        xT[:, kc, i5 * 512 : (i5 + 1) * 512], pt[:]
                    )

            # ---- phase 2: q/k projection, feature-major ----
            # m = 0,1: q head pairs (0,1), (2,3); m = 2,3: k head pairs
            for m in range(4):
                for i5 in range(NI5):
                    pp = ps_proj.tile([128, 512], F32, tag="pp")
                    for kc in range(4):
                        nc.tensor.matmul(
                            pp[:],
                            wT[:, kc, m * 128 : (m + 1) * 128],
                            xT[:, kc, i5 * 512 : (i5 + 1) * 512],
                            start=(kc == 0),
                            stop=(kc == 3),
                        )
                    if m < 2:
                        nc.vector.tensor_copy(
                            qcT[:, m, 1 + i5 * 512 : 1 + (i5 + 1) * 512], pp[:]
                        )
                        if i5 == 0:
                            nc.vector.tensor_copy(qcT[:, m, 0:1], pp[:, 0:1])
                    else:
                        nc.vector.tensor_copy(
                            kT[:, m - 2, i5 * 512 : (i5 + 1) * 512], pp[:]
                        )

            # ---- phase 3: v projection, token-major ----
            for jc in range(NI):
                pv = ps_proj.tile([128, 512], F32, tag="pp")
                for kc in range(4):
                    nc.tensor.matmul(
                        pv[:, :RV],
                        xT[:, kc, jc * 128 : (jc + 1) * 128],
                        wT[:, kc, 2 * RV : 3 * RV],
                        start=(kc == 0),
                        stop=(kc == 3),
                    )
                nc.vector.tensor_copy(
                    v1[:, jc, :].rearrange("p (h e) -> p h e", e=65)[:, :, 0:64],
                    pv[:, :RV].rearrange("p (h e) -> p h e", e=64),
                )

        # ---- phase 4: attention ----
        with tc.tile_pool(name="ps_st", bufs=2, space="PSUM") as ps_st, tc.tile_pool(
            name="ps_ot", bufs=2, space="PSUM"
        ) as ps_ot, tc.tile_pool(
            name="ps_tro", bufs=2, space="PSUM"
        ) as ps_tro, tc.tile_pool(name="e_sb", bufs=3) as e_pool, tc.tile_pool(
            name="ot_sb", bufs=2
        ) as ot_pool, tc.tile_pool(name="rs", bufs=4) as rs_pool:
            for hp in range(2):
                for i5 in range(NI5):
                    ot_ps = [
                        ps_ot.tile([65, 512], F32, tag="ot", name=f"ot{hp}_{i5}_{h}")
                        for h in range(2)
                    ]
                    for jc in range(NI):
                        st = ps_st.tile([128, 1024], F32, tag="st")
                        # two heads row-packed: head 2hp on array rows 0-63,
                        # head 2hp+1 on rows 64-127
                        nc.tensor.matmul(
                            st[:, 0:512],
                            kT[0:64, hp, jc * 128 : (jc + 1) * 128],
                            qcT[0:64, hp, i5 * 512 : (i5 + 1) * 512],
                            start=True,
                            stop=True,
                            tile_position=(0, 0),
                        )
                        nc.tensor.matmul(
                            st[:, 512:1024],
                            kT[64:128, hp, jc * 128 : (jc + 1) * 128],
                            qcT[64:128, hp, i5 * 512 : (i5 + 1) * 512],
                            start=True,
                            stop=True,
                            tile_position=(64, 0),
                        )
                        et = e_pool.tile([128, 1024], BF16, tag="et")
                        nc.scalar.activation(et[:], st[:], EXP, scale=SCALE)
                        for h in range(2):
                            ha = hp * 2 + h
                            nc.tensor.matmul(
                                ot_ps[h][:],
                                v1[:, jc, ha * 65 : (ha + 1) * 65],
                                et[:, h * 512 : (h + 1) * 512],
                                start=(jc == 0),
                                stop=(jc == NI - 1),
                            )
                    for h in range(2):
                        ha = hp * 2 + h
                        ots = ot_pool.tile([65, 512], F32, tag="ots")
                        nc.vector.tensor_copy(ots[:], ot_ps[h][:])
                        for t in range(4):
                            ic = i5 * 4 + t
                            tr = ps_tro.tile([128, 65], F32, tag="tr")
                            nc.tensor.transpose(
                                tr[:], ots[:, t * 128 : (t + 1) * 128], identf[0:65, 0:65]
                            )
                            rs = rs_pool.tile([128, 1], F32, tag="rs")
                            nc.vector.reciprocal(rs[:], tr[:, 64:65])
                            nc.vector.tensor_scalar_mul(
                                out_sb[:, ic, ha * 64 : (ha + 1) * 64],
                                tr[:, 0:64],
                                rs[:],
                            )

            for ic in range(NI):
                nc.sync.dma_start(o_d[ic * 128 : (ic + 1) * 128, :], out_sb[:, ic, :])

    return nc


def make_nc(n_tok: int = N) -> bass.Bass:
    nc = bacc.Bacc("TRN2", target_bir_lowering=False, debug=False)
    build_kernel(nc, n_tok=n_tok)
    nc.compile()
    return nc


def shard_inputs(x: np.ndarray, W_qkv: np.ndarray) -> list[dict]:
    """Core c = (b, g): b = c // 2, g = c % 2 (heads 4g..4g+3)."""
    in_maps = []
    for c in range(NCORES):
        b, g = divmod(c, 2)
        r0 = g * RV
        w_shard = np.concatenate(
            [
                W_qkv[r0 : r0 + RV],
                W_qkv[512 + r0 : 512 + r0 + RV],
                W_qkv[1024 + r0 : 1024 + r0 + RV],
            ],
            axis=0,
        )
        in_maps.append(
            {
                "x": np.ascontiguousarray(x[b], dtype=np.float32),
                "w": np.ascontiguousarray(w_shard, dtype=np.float32),
            }
        )
    return in_maps


def gather_outputs(results: list[dict]) -> np.ndarray:
    out = np.empty((B, N, H * DH), dtype=np.float32)
    for c in range(NCORES):
        b, g = divmod(c, 2)
        out[b, :, g * RV : (g + 1) * RV] = results[c]["o"]
    return out


_CACHED_NC = None


def kernel(x: np.ndarray, W_qkv: np.ndarray) -> np.ndarray:
    global _CACHED_NC
    from concourse.bass_utils import run_bass_kernel_spmd

    if _CACHED_NC is None:
        _CACHED_NC = make_nc()
    in_maps = shard_inputs(np.asarray(x), np.asarray(W_qkv))
    res = run_bass_kernel_spmd(_CACHED_NC, in_maps, core_ids=list(range(NCORES)))
    return gather_outputs(res.results)


if __name__ == "__main__":
    rng = np.random.default_rng(0)
    x = rng.standard_normal((B, N, D), dtype=np.float32)
    w = (rng.standard_normal((3 * H * DH, D), dtype=np.float32) * 0.02).astype(
        np.float32
    )
    out = kernel(x, w)
    print(out.shape, out.dtype)
